# revision 1
# baseline (speedup 1.0000x reference)
"""AttentionDAF Trainium2 kernel — data-parallel over batch across 8 NeuronCores.

Reference computation (per batch element, c=inputs (512,768), q=states (512,768)):
    cq[i,j] = sum_h c[i,h]*wcq[h]*q[j,h]  (+biases)
    s = s_c[:,None] + s_q[None,:] + cq + mask
    a = softmax_j(s);  c2q = a @ q
    b = softmax_i(max_j s);  q2c = b @ c (broadcast over rows)
    x = [c, c2q, c*c2q, c*q2c]  (512, 3072)
    y = relu(x @ wa^T + wa_b) + c;  out = layernorm(y)*g + b

Key algebraic facts used:
  - softmax_j(s) is invariant to per-row constants: s_c and ALL linear biases drop
    out of `a`. Only s0 = cq0 + s_q (+mask) matters, with cq0 = (c*wcq) @ q^T.
  - b = softmax_i(max_j s) is invariant to global constants: biases drop; only
    m[i] = s_c[i] + max_j(s0[i,:]) matters.
Per-core work: 2 batch elements, no collectives. Matmuls in bf16 (f32 PSUM accum).
Host pre-transposes/casts inputs (layout prep only; all FLOPs on device).
"""
import sys
from contextlib import ExitStack

if "/opt/trn_rl_repo" not in sys.path:
    sys.path.insert(0, "/opt/trn_rl_repo")

import numpy as np
import ml_dtypes

from concourse import bacc
import concourse.bacc as bacc_mod
import concourse.hw_specs as hw_specs
import concourse.bass as bass
import concourse.tile as tile
import concourse.mybir as mybir
from concourse.bass_utils import run_bass_kernel_spmd
from concourse.masks import make_identity

F32 = mybir.dt.float32
BF16 = mybir.dt.bfloat16
AF = mybir.ActivationFunctionType
X = mybir.AxisListType.X
MULT = mybir.AluOpType.mult
SUB = mybir.AluOpType.subtract
MAXOP = mybir.AluOpType.max

B, CL, QL, H = 16, 512, 512, 768
N_CORES = 8
BPC = B // N_CORES      # batch elements per core
PC = CL // 128          # i-chunks (c rows)
QC = QL // 128          # j-chunks (q rows)
HC = H // 128           # h-chunks
FC = 4 * HC             # f-chunks of concat feature dim (3072)
LN_EPS = 1e-5
BF = ml_dtypes.bfloat16

# All activation funcs we use (Exp, Ln, Relu, Copy, Identity, Square) live in
# the "natural_log_exp_and_others" table set. bass's table-load inserter picks
# the first set containing each func, which thrashes between exp_and_others and
# natural_log (2.7us per switch). Blank out every other set's advertised
# contents so exactly one load is emitted; set ids keep matching act_info.json.
_ORIG_GAT = hw_specs.get_activation_tables


def _single_set_tables(arch):
    t = _ORIG_GAT(arch)
    return {
        name: (funcs if name == "natural_log_exp_and_others" else set())
        for name, funcs in t.items()
    }


bacc_mod.get_activation_tables = _single_set_tables


def build_kernel(use_mask: bool, trivial_ln: bool, reps: int = 1,
                 skip_stages: frozenset = frozenset()):
    """skip_stages: subset of {"softmax","front","big","epilogue"} for
    timeline/HW ablation probes (output is garbage when non-empty)."""
    nc = bacc.Bacc("TRN2", target_bir_lowering=False, debug=False)

    # ---- DRAM I/O (per-core shard shapes) ----
    d_c32 = nc.dram_tensor("c32", [BPC, CL, H], F32, kind="ExternalInput")
    d_cbf = nc.dram_tensor("cbf", [BPC, CL, H], BF16, kind="ExternalInput")
    d_cT = nc.dram_tensor("cT", [BPC, H, CL], BF16, kind="ExternalInput")
    d_qT = nc.dram_tensor("qT", [BPC, H, QL], BF16, kind="ExternalInput")
    d_qn = nc.dram_tensor("qn", [BPC, QL, H], BF16, kind="ExternalInput")
    d_wcq = nc.dram_tensor("wcq", [128, HC], F32, kind="ExternalInput")
    d_wc = nc.dram_tensor("wc", [128, HC], BF16, kind="ExternalInput")
    d_wq = nc.dram_tensor("wq", [128, HC], BF16, kind="ExternalInput")
    d_waT = nc.dram_tensor("waT", [128, FC, H], BF16, kind="ExternalInput")
    d_wab = nc.dram_tensor("wab", [1, H], BF16, kind="ExternalInput")
    if use_mask:
        d_mask = nc.dram_tensor("mask", [BPC, CL, QL], F32, kind="ExternalInput")
    if not trivial_ln:
        d_lng = nc.dram_tensor("lng", [H], F32, kind="ExternalInput")
        d_lnb = nc.dram_tensor("lnb", [H], F32, kind="ExternalInput")
    d_out = nc.dram_tensor("out", [BPC, CL, H], F32, kind="ExternalOutput")

    with tile.TileContext(nc) as tc, ExitStack() as ctx:
        consts = ctx.enter_context(tc.tile_pool(name="consts", bufs=1))
        p_in32 = ctx.enter_context(tc.tile_pool(name="in32", bufs=2))
        p_inbf = ctx.enter_context(tc.tile_pool(name="inbf", bufs=2))
        p_work = ctx.enter_context(tc.tile_pool(name="work", bufs=1))
        p_xmat = ctx.enter_context(tc.tile_pool(name="xmat", bufs=2))
        p_small = ctx.enter_context(tc.tile_pool(name="small", bufs=2))
        p_y = ctx.enter_context(tc.tile_pool(name="ypool", bufs=2))
        # PSUM budget is 8 banks of [128 x 512 f32]:
        #   ps_mm  "mm"  [128,512] x2 = 2 banks (s0 / A^T / c2q^T stages)
        #   ps_row "row" [<=128,<=512] x2 = 2 banks (all small/row tiles)
        #   ps_big "big" [128,768] x2 = 4 banks (final matmul)
        ps_mm = ctx.enter_context(tc.tile_pool(name="ps_mm", bufs=3, space="PSUM"))
        ps_row = ctx.enter_context(tc.tile_pool(name="ps_row", bufs=1, space="PSUM"))
        ps_big = ctx.enter_context(tc.tile_pool(name="ps_big", bufs=2, space="PSUM"))

        # ---- constants (once per core) ----
        waT = consts.tile([128, FC, H], BF16)
        nc.scalar.dma_start(waT[:], d_waT.ap()[:])
        wcq_c = consts.tile([128, HC], F32)
        nc.sync.dma_start(wcq_c[:], d_wcq.ap()[:])
        wc_c = consts.tile([128, HC], BF16)
        nc.sync.dma_start(wc_c[:], d_wc.ap()[:])
        wq_c = consts.tile([128, HC], BF16)
        nc.sync.dma_start(wq_c[:], d_wq.ap()[:])
        wab_pad = consts.tile([128, H], BF16)
        nc.vector.memset(wab_pad[:], 0.0)
        nc.sync.dma_start(wab_pad[0:1, :], d_wab.ap()[:])
        id_bf = consts.tile([128, 128], BF16)
        make_identity(nc, id_bf[:])
        id_f32 = consts.tile([128, 128], F32)
        make_identity(nc, id_f32[:])
        nid_f32 = consts.tile([128, 128], F32)
        nc.gpsimd.memset(nid_f32[:], 0.0)
        nc.gpsimd.affine_select(
            out=nid_f32[:], in_=nid_f32[:],
            compare_op=mybir.AluOpType.not_equal, fill=-1.0,
            base=0, pattern=[[-1, 128]], channel_multiplier=1,
        )
        eps_t = consts.tile([128, 1], F32)
        nc.vector.memset(eps_t[:], LN_EPS)
        ones_t = consts.tile([128, 128], BF16)
        nc.vector.memset(ones_t[:], 0.0)
        nc.vector.memset(ones_t[0:1, :], 1.0)
        if not trivial_ln:
            g_bc = consts.tile([128, H], F32)
            nc.sync.dma_start(
                g_bc[:],
                bass.AP(tensor=d_lng, offset=0, ap=[[0, 128], [1, H]]),
            )
            b_bc = consts.tile([128, H], F32)
            nc.sync.dma_start(
                b_bc[:],
                bass.AP(tensor=d_lnb, offset=0, ap=[[0, 128], [1, H]]),
            )

        rep_ctx = tc.For_i(0, reps, 1) if reps > 1 else None
        if rep_ctx is not None:
            rep_ctx.__enter__()
        for b in range(BPC):
            # ---- per-batch loads ----
            qT = p_inbf.tile([128, HC, QL], BF16, tag="qT")
            nc.sync.dma_start(qT[:], d_qT.ap()[b].rearrange("(o p) j -> p o j", p=128))
            cT = p_inbf.tile([128, HC, CL], BF16, tag="cT")
            nc.sync.dma_start(cT[:], d_cT.ap()[b].rearrange("(o p) i -> p o i", p=128))
            qn = p_inbf.tile([128, QC, H], BF16, tag="qn")
            nc.scalar.dma_start(qn[:], d_qn.ap()[b].rearrange("(o p) h -> p o h", p=128))
            cbf = p_inbf.tile([128, PC, H], BF16, tag="cbf")
            nc.scalar.dma_start(cbf[:], d_cbf.ap()[b].rearrange("(o p) h -> p o h", p=128))
            c32 = p_in32.tile([128, PC, H], F32, tag="c32")
            nc.scalar.dma_start(c32[:], d_c32.ap()[b].rearrange("(o p) h -> p o h", p=128))
            if use_mask:
                mk = p_inbf.tile([128, PC, QL], F32, tag="mask")
                nc.sync.dma_start(
                    mk[:], d_mask.ap()[b].rearrange("(o p) j -> p o j", p=128)
                )

            if "front" not in skip_stages:
                # ---- c_scaled^T = cT * wcq (per-partition scalar per h-chunk) ----
                csT = p_work.tile([128, HC, CL], BF16, tag="csT")
                for hc in range(HC):
                    nc.vector.tensor_scalar_mul(csT[:, hc], cT[:, hc], wcq_c[:, hc : hc + 1])

                # ---- s_q row -> rank-1 rhs (rhs2 row0), rest zeros ----
                rhs2 = p_work.tile([128, QL], BF16, tag="rhs2")
                nc.vector.memset(rhs2[:], 0.0)
                sq_ps = ps_row.tile([1, QL], F32, tag="row")
                for hc in range(HC):
                    nc.tensor.matmul(
                        sq_ps[:], lhsT=wq_c[:, hc : hc + 1], rhs=qT[:, hc],
                        start=(hc == 0), stop=(hc == HC - 1),
                    )
                nc.scalar.copy(rhs2[0:1, :], sq_ps[:])

                # ---- s0 = cq0 + s_q (+mask); rowmax; E = exp(s0 - rowmax); rowsum ----
                E = p_work.tile([128, PC, QL], BF16, tag="E")
                nrm = p_small.tile([128, PC], F32, tag="nrm")   # -rowmax
                rs = p_small.tile([128, PC], F32, tag="rs")     # rowsum of E
                for ic in range(PC):
                    s0 = ps_mm.tile([128, QL], F32, tag="mm")
                    for hc in range(HC):
                        nc.tensor.matmul(
                            s0[:], lhsT=csT[:, hc, ic * 128 : (ic + 1) * 128],
                            rhs=qT[:, hc], start=(hc == 0), stop=False,
                        )
                    nc.tensor.matmul(s0[:], lhsT=ones_t[:], rhs=rhs2[:], start=False, stop=True)
                    if use_mask:
                        nc.vector.tensor_add(s0[:], s0[:], mk[:, ic])
                    if "softmax" in skip_stages:
                        continue
                    nc.vector.tensor_reduce(
                        out=nrm[:, ic : ic + 1], in_=s0[:], axis=X, op=MAXOP,
                        negate=True,
                    )
                    nc.scalar.activation(
                        out=E[:, ic], in_=s0[:], func=AF.Exp,
                        bias=nrm[:, ic : ic + 1], scale=1.0,
                        accum_out=rs[:, ic : ic + 1],
                    )

                # ---- 1/rowsum, diag blocks, A^T = E^T * diag (transpose+normalize) ----
                rr = p_small.tile([128, PC], F32, tag="rr")
                diag = p_work.tile([128, PC, 128], BF16, tag="diag")
                for ic in range(PC):
                    nc.vector.reciprocal(rr[:, ic : ic + 1], rs[:, ic : ic + 1])
                    nc.vector.tensor_scalar_mul(diag[:, ic], id_bf[:], rr[:, ic : ic + 1])
                AT = p_work.tile([128, QC, CL], BF16, tag="AT")
                for jc in range(QC):
                    at_ps = ps_mm.tile([128, CL], F32, tag="mm")
                    for ic in range(PC):
                        nc.tensor.matmul(
                            at_ps[:, ic * 128 : (ic + 1) * 128],
                            lhsT=E[:, ic, jc * 128 : (jc + 1) * 128],
                            rhs=diag[:, ic], start=True, stop=True,
                        )
                    nc.vector.tensor_copy(AT[:, jc], at_ps[:])

                # ---- c2q^T (h-part) + xc = (c*c2q)^T ----
                c2qT = p_xmat.tile([128, HC, CL], BF16, tag="c2qT")
                xc = p_xmat.tile([128, HC, CL], BF16, tag="xc")
                for hc in range(HC):
                    cq_ps = ps_mm.tile([128, CL], F32, tag="mm")
                    for jc in range(QC):
                        nc.tensor.matmul(
                            cq_ps[:], lhsT=qn[:, jc, hc * 128 : (hc + 1) * 128],
                            rhs=AT[:, jc], start=(jc == 0), stop=(jc == QC - 1),
                        )
                    nc.scalar.copy(c2qT[:, hc], cq_ps[:])
                    nc.vector.tensor_tensor(
                        xc[:, hc], cT[:, hc], c2qT[:, hc], op=MULT
                    )

                # ---- b path: m = s_c + rowmax0 (row form), softmax over free dim ----
                m_ps = ps_row.tile([1, CL], F32, tag="row")
                for hc in range(HC):
                    nc.tensor.matmul(
                        m_ps[:], lhsT=wc_c[:, hc : hc + 1], rhs=cT[:, hc],
                        start=(hc == 0), stop=False,
                    )
                for ic in range(PC):
                    nc.tensor.matmul(
                        m_ps[0:1, ic * 128 : (ic + 1) * 128],
                        lhsT=nrm[:, ic : ic + 1], rhs=nid_f32[:],
                        start=False, stop=(ic == PC - 1),
                    )
                nmax = p_small.tile([1, 1], F32, tag="nmax")
                nc.vector.tensor_reduce(out=nmax[:], in_=m_ps[:], axis=X, op=MAXOP, negate=True)
                eb = p_small.tile([1, CL], F32, tag="eb")
                bS = p_small.tile([1, 1], F32, tag="bS")
                nc.scalar.activation(
                    out=eb[:], in_=m_ps[:], func=AF.Exp, bias=nmax[:], scale=1.0,
                    accum_out=bS[:],
                )
                rbS = p_small.tile([1, 1], F32, tag="rbS")
                nc.vector.reciprocal(rbS[:], bS[:])
                bnorm = p_small.tile([1, CL], F32, tag="bnorm")
                nc.vector.tensor_scalar_mul(bnorm[:], eb[:], rbS[:])
                # transpose b row -> columns [128, PC] via K=1 matmuls
                bc_ps = ps_row.tile([128, PC], F32, tag="row")
                for ic in range(PC):
                    nc.tensor.matmul(
                        bc_ps[:, ic : ic + 1],
                        lhsT=bnorm[0:1, ic * 128 : (ic + 1) * 128],
                        rhs=id_f32[0:1, 0:1], start=True, stop=True,
                    )
                b_cols = p_small.tile([128, PC], BF16, tag="b_cols")
                nc.scalar.copy(b_cols[:], bc_ps[:])

                # ---- q2c row = b @ c  -> columns (h-part) -> xq = (c*q2c)^T ----
                q2c_sb = p_small.tile([1, H], F32, tag="q2c_sb")
                for n0, nw in ((0, 512), (512, 256)):
                    qp = ps_row.tile([1, nw], F32, tag="row")
                    for ic in range(PC):
                        nc.tensor.matmul(
                            qp[:],
                            lhsT=b_cols[:, ic : ic + 1],
                            rhs=cbf[:, ic, n0 : n0 + nw],
                            start=(ic == 0), stop=(ic == PC - 1),
                        )
                    nc.scalar.copy(q2c_sb[0:1, n0 : n0 + nw], qp[:])
                qcc_ps = ps_row.tile([128, HC], F32, tag="row")
                for hc in range(HC):
                    nc.tensor.matmul(
                        qcc_ps[:, hc : hc + 1],
                        lhsT=q2c_sb[0:1, hc * 128 : (hc + 1) * 128],
                        rhs=id_f32[0:1, 0:1], start=True, stop=True,
                    )
                q2c_c = p_small.tile([128, HC], F32, tag="q2c_c")
                nc.scalar.copy(q2c_c[:], qcc_ps[:])
                # Fold the (c*q2c) concat component into the c-component weights:
                #   sum_f cT[f,i]*q2c[f]*wa4T[f,ho] == c @ (diag(q2c) wa4T)
                # so big-matmul uses merged = wa1T + q2c (.) wa4T for comp 0.
                merged = p_work.tile([128, HC, H], BF16, tag="merged")
                for hc in range(HC):
                    nc.vector.tensor_scalar_mul(
                        merged[:, hc], waT[:, 3 * HC + hc], q2c_c[:, hc : hc + 1]
                    )
                    nc.vector.tensor_add(merged[:, hc], merged[:, hc], waT[:, hc])

            if "big" not in skip_stages:
                # ---- big matmul: y0 = x @ wa^T + wa_b; relu; +c; layernorm ----
                if "front" in skip_stages:
                    comps = ((cT, 0, None), (cT, 1, None), (cT, 2, None))
                else:
                    # (tile, waT comp index, rhs override); merged last — its
                    # weights depend on the b-path and arrive latest.
                    comps = ((c2qT, 1, None), (xc, 2, None), (cT, 0, merged))
                yt = p_y.tile([128, PC, H], F32, tag="y")
                for ic in range(PC):
                    big_ps = ps_big.tile([128, H], F32, tag="big")
                    k = 0
                    for comp, wci, rhs_src in comps:
                        for hc in range(HC):
                            rhs3 = (
                                rhs_src[:, hc] if rhs_src is not None
                                else waT[:, wci * HC + hc]
                            )
                            for n0, nw in ((0, 512), (512, 256)):
                                nc.tensor.matmul(
                                    big_ps[:, n0 : n0 + nw],
                                    lhsT=comp[:, hc, ic * 128 : (ic + 1) * 128],
                                    rhs=rhs3[:, n0 : n0 + nw],
                                    start=(k == 0), stop=False,
                                    skip_group_check=True,
                                )
                            k += 1
                    for n0, nw in ((0, 512), (512, 256)):
                        nc.tensor.matmul(
                            big_ps[:, n0 : n0 + nw], lhsT=ones_t[:],
                            rhs=wab_pad[:, n0 : n0 + nw], start=False, stop=True,
                            skip_group_check=True,
                        )
                    if "epilogue" in skip_stages:
                        continue
                    nc.scalar.activation(out=yt[:, ic], in_=big_ps[:], func=AF.Relu)
                    nc.vector.tensor_add(yt[:, ic], yt[:, ic], c32[:, ic])
                    stats = p_small.tile([128, 3, 6], F32, tag="stats")
                    for sg in range(3):
                        nc.vector.bn_stats(
                            out=stats[:, sg], in_=yt[:, ic, sg * 256 : (sg + 1) * 256]
                        )
                    mv = p_small.tile([128, 2], F32, tag="mv")
                    nc.vector.bn_aggr(out=mv[:], in_=stats[:])
                    lnv = p_small.tile([128, 1], F32, tag="lnv")
                    nc.scalar.activation(
                        out=lnv[:], in_=mv[:, 1:2], func=AF.Ln, bias=eps_t[:], scale=1.0
                    )
                    rstd = p_small.tile([128, 1], F32, tag="rstd")
                    nc.scalar.activation(out=rstd[:], in_=lnv[:], func=AF.Exp, scale=-0.5)
                    nc.vector.tensor_scalar(
                        out=yt[:, ic], in0=yt[:, ic],
                        scalar1=mv[:, 0:1], scalar2=rstd[:],
                        op0=SUB, op1=MULT,
                    )
                    if not trivial_ln:
                        nc.vector.tensor_tensor(yt[:, ic], yt[:, ic], g_bc[:], op=MULT)
                        nc.vector.tensor_add(yt[:, ic], yt[:, ic], b_bc[:])
                    nc.sync.dma_start(
                        d_out.ap()[b].rearrange("(o p) h -> p o h", p=128)[:, ic], yt[:, ic]
                    )

        if rep_ctx is not None:
            rep_ctx.__exit__(None, None, None)

    nc.compile()
    return nc


_KERNEL_CACHE = {}


def get_kernel(use_mask: bool, trivial_ln: bool):
    key = (use_mask, trivial_ln)
    if key not in _KERNEL_CACHE:
        _KERNEL_CACHE[key] = build_kernel(use_mask, trivial_ln)
    return _KERNEL_CACHE[key]


def prep_inputs(inputs):
    """Host-side layout prep: shard over batch, transpose/cast, weight reshape."""
    c = np.ascontiguousarray(np.asarray(inputs["inputs"], dtype=np.float32))
    q = np.ascontiguousarray(np.asarray(inputs["states"], dtype=np.float32))
    mask = np.asarray(inputs["attention_mask"], dtype=np.float32)[:, 0]
    use_mask = bool(np.any(mask))
    ln_g = np.asarray(inputs["ln_g"], dtype=np.float32)
    ln_b = np.asarray(inputs["ln_b"], dtype=np.float32)
    trivial_ln = bool(np.all(ln_g == 1.0) and np.all(ln_b == 0.0))

    cbf = c.astype(BF)
    qn = q.astype(BF)
    cT = np.ascontiguousarray(c.transpose(0, 2, 1)).astype(BF)
    qT = np.ascontiguousarray(q.transpose(0, 2, 1)).astype(BF)

    wcq_cols = np.ascontiguousarray(
        np.asarray(inputs["wcq_w"], np.float32)[0].reshape(HC, 128).T
    )
    wc_cols = np.ascontiguousarray(
        np.asarray(inputs["wc_w"], np.float32)[0].reshape(HC, 128).T
    ).astype(BF)
    wq_cols = np.ascontiguousarray(
        np.asarray(inputs["wq_w"], np.float32)[0].reshape(HC, 128).T
    ).astype(BF)
    waT = np.ascontiguousarray(
        np.asarray(inputs["wa_w"], np.float32).T.reshape(FC, 128, H).transpose(1, 0, 2)
    ).astype(BF)
    wab = np.asarray(inputs["wa_b"], np.float32).reshape(1, H).astype(BF)

    in_maps = []
    for k in range(N_CORES):
        sl = slice(k * BPC, (k + 1) * BPC)
        m = {
            "c32": c[sl],
            "cbf": cbf[sl],
            "cT": cT[sl],
            "qT": qT[sl],
            "qn": qn[sl],
            "wcq": wcq_cols,
            "wc": wc_cols,
            "wq": wq_cols,
            "waT": waT,
            "wab": wab,
        }
        if use_mask:
            m["mask"] = np.ascontiguousarray(mask[sl])
        if not trivial_ln:
            m["lng"] = ln_g
            m["lnb"] = ln_b
        in_maps.append(m)
    return in_maps, use_mask, trivial_ln


def kernel(**inputs) -> np.ndarray:
    in_maps, use_mask, trivial_ln = prep_inputs(inputs)
    nc = get_kernel(use_mask, trivial_ln)
    res = run_bass_kernel_spmd(nc, in_maps, core_ids=list(range(N_CORES)))
    return np.concatenate([res.results[k]["out"] for k in range(N_CORES)], axis=0)



# revision 10
# speedup vs baseline: 1.0828x; 1.0828x over previous
"""AttentionDAF Trainium2 kernel — data-parallel over batch across 8 NeuronCores.

Reference computation (per batch element, c=inputs (512,768), q=states (512,768)):
    cq[i,j] = sum_h c[i,h]*wcq[h]*q[j,h]  (+biases)
    s = s_c[:,None] + s_q[None,:] + cq + mask
    a = softmax_j(s);  c2q = a @ q
    b = softmax_i(max_j s);  q2c = b @ c (broadcast over rows)
    x = [c, c2q, c*c2q, c*q2c]  (512, 3072)
    y = relu(x @ wa^T + wa_b) + c;  out = layernorm(y)*g + b

Key algebraic facts used:
  - softmax_j(s) is invariant to per-row constants: s_c and ALL linear biases drop
    out of `a`. Only s0 = cq0 + s_q (+mask) matters, with cq0 = (c*wcq) @ q^T.
  - b = softmax_i(max_j s) is invariant to global constants: biases drop; only
    m[i] = s_c[i] + max_j(s0[i,:]) matters.
Per-core work: 2 batch elements, no collectives. Matmuls in bf16 (f32 PSUM accum).
Host pre-transposes/casts inputs (layout prep only; all FLOPs on device).

Engine assignment notes (v2):
  - s_q is computed pre-broadcast on PE via a free-dim-replicated weight tile
    (wq_rep), then added into the s0 PSUM by GpSimd — no per-row bias matmuls.
  - s_c / the b-softmax run fully in column form: GpSimd computes the c.wc dot
    rows via an accumulating scalar_tensor_tensor, the softmax normalizer via
    partition_all_reduce. No M=1 PE matmuls, no b transposes.
  - The big-matmul bias is added by GpSimd from a partition-broadcast bias
    tile; relu+residual is a single fused DVE scalar_tensor_tensor.
  - Residual uses the bf16 c copy (cbf); the f32 c input is dropped entirely.
"""
import sys
from contextlib import ExitStack

if "/opt/trn_rl_repo" not in sys.path:
    sys.path.insert(0, "/opt/trn_rl_repo")

import numpy as np
import ml_dtypes

from concourse import bacc
import concourse.bacc as bacc_mod
import concourse.hw_specs as hw_specs
import concourse.bass as bass
import concourse.bass_isa as bass_isa
import concourse.tile as tile
import concourse.mybir as mybir
from concourse.bass_utils import run_bass_kernel_spmd
from concourse.masks import make_identity

F32 = mybir.dt.float32
BF16 = mybir.dt.bfloat16
AF = mybir.ActivationFunctionType
X = mybir.AxisListType.X
ADD = mybir.AluOpType.add
MULT = mybir.AluOpType.mult
SUB = mybir.AluOpType.subtract
MAXOP = mybir.AluOpType.max

B, CL, QL, H = 16, 512, 512, 768
N_CORES = 8
BPC = B // N_CORES      # batch elements per core
PC = CL // 128          # i-chunks (c rows)
QC = QL // 128          # j-chunks (q rows)
HC = H // 128           # h-chunks
FC = 4 * HC             # f-chunks of concat feature dim (3072)
LN_EPS = 1e-5
BF = ml_dtypes.bfloat16

# All activation funcs we use (Exp, Ln, Copy, Identity) live in the
# "natural_log_exp_and_others" table set. bass's table-load inserter picks
# the first set containing each func, which thrashes between exp_and_others and
# natural_log (2.7us per switch). Blank out every other set's advertised
# contents so exactly one load is emitted; set ids keep matching act_info.json.
_ORIG_GAT = hw_specs.get_activation_tables


def _single_set_tables(arch):
    t = _ORIG_GAT(arch)
    return {
        name: (funcs if name == "natural_log_exp_and_others" else set())
        for name, funcs in t.items()
    }


bacc_mod.get_activation_tables = _single_set_tables


def build_kernel(use_mask: bool, trivial_ln: bool, reps: int = 1,
                 skip_stages: frozenset = frozenset()):
    """skip_stages: subset of {"softmax","front","big","epilogue"} for
    timeline/HW ablation probes (output is garbage when non-empty)."""
    nc = bacc.Bacc("TRN2", target_bir_lowering=False, debug=False)

    # ---- DRAM I/O (per-core shard shapes) ----
    d_cbf = nc.dram_tensor("cbf", [BPC, CL, H], BF16, kind="ExternalInput")
    d_cT = nc.dram_tensor("cT", [BPC, H, CL], BF16, kind="ExternalInput")
    d_qT = nc.dram_tensor("qT", [BPC, H, QL], BF16, kind="ExternalInput")
    d_qn = nc.dram_tensor("qn", [BPC, QL, H], BF16, kind="ExternalInput")
    d_wcq = nc.dram_tensor("wcq", [128, HC], F32, kind="ExternalInput")
    d_wc = nc.dram_tensor("wc", [1, H], BF16, kind="ExternalInput")
    d_wq = nc.dram_tensor("wq", [128, HC], F32, kind="ExternalInput")
    d_waT = nc.dram_tensor("waT", [128, FC, H], BF16, kind="ExternalInput")
    d_wab = nc.dram_tensor("wab", [1, H], F32, kind="ExternalInput")
    if use_mask:
        d_mask = nc.dram_tensor("mask", [BPC, CL, QL], F32, kind="ExternalInput")
    if not trivial_ln:
        d_lng = nc.dram_tensor("lng", [H], F32, kind="ExternalInput")
        d_lnb = nc.dram_tensor("lnb", [H], F32, kind="ExternalInput")
    d_out = nc.dram_tensor("out", [BPC, CL, H], F32, kind="ExternalOutput")

    RADD = bass_isa.ReduceOp.add

    with tile.TileContext(nc) as tc, ExitStack() as ctx:
        consts = ctx.enter_context(tc.tile_pool(name="consts", bufs=1))
        p_inbf = ctx.enter_context(tc.tile_pool(name="inbf", bufs=2))
        p_work = ctx.enter_context(tc.tile_pool(name="work", bufs=1))
        p_xmat = ctx.enter_context(tc.tile_pool(name="xmat", bufs=2))
        p_small = ctx.enter_context(tc.tile_pool(name="small", bufs=2))
        p_y = ctx.enter_context(tc.tile_pool(name="ypool", bufs=2))
        # PSUM budget is 8 banks of [128 x 512 f32]:
        #   ps_mm  "mm"  [128,512] x3 = 3 banks (s0 / A^T / c2q^T stages)
        #   ps_aux "aux" [<=128,<=512] x1 = 1 bank (sq bcast, q2c row/col)
        #   ps_big "big" [128,768] x2 = 4 banks (final matmul)
        ps_mm = ctx.enter_context(tc.tile_pool(name="ps_mm", bufs=3, space="PSUM"))
        ps_aux = ctx.enter_context(tc.tile_pool(name="ps_aux", bufs=1, space="PSUM"))
        ps_big = ctx.enter_context(tc.tile_pool(name="ps_big", bufs=2, space="PSUM"))

        # ---- constants (once per core; DMAs on the gpsimd/SWDGE queue so
        # they never delay the per-batch loads on the SP/ACT queues) ----
        waT = consts.tile([128, FC, H], BF16)
        nc.gpsimd.dma_start(waT[:], d_waT.ap()[:])
        wcq_c = consts.tile([128, HC], F32)
        nc.gpsimd.dma_start(wcq_c[:], d_wcq.ap()[:])
        wq_c = consts.tile([128, HC], F32)
        nc.gpsimd.dma_start(wq_c[:], d_wq.ap()[:])
        wc_stage = consts.tile([1, H], BF16)
        nc.gpsimd.dma_start(wc_stage[:], d_wc.ap()[:])
        wab_stage = consts.tile([1, H], F32)
        nc.gpsimd.dma_start(wab_stage[:], d_wab.ap()[:])
        wc_bc = consts.tile([128, H], BF16)
        nc.gpsimd.partition_broadcast(wc_bc[:], wc_stage[:])
        wab_bc = consts.tile([128, H], F32)
        nc.gpsimd.partition_broadcast(wab_bc[:], wab_stage[:])
        ones_bf = consts.tile([128, 128], BF16)
        nc.vector.memset(ones_bf[:], 1.0)
        wq_rep = consts.tile([128, HC, 128], BF16)
        for hc in range(HC):
            nc.vector.tensor_scalar_mul(wq_rep[:, hc], ones_bf[:], wq_c[:, hc : hc + 1])
        id_bf = consts.tile([128, 128], BF16)
        make_identity(nc, id_bf[:])
        id_f32 = consts.tile([128, 128], F32)
        make_identity(nc, id_f32[:])
        eps_t = consts.tile([128, 1], F32)
        nc.vector.memset(eps_t[:], LN_EPS)
        nb3_t = consts.tile([128, 1], F32)
        nc.vector.memset(nb3_t[:], -3.0)
        if not trivial_ln:
            g_bc = consts.tile([128, H], F32)
            nc.gpsimd.dma_start(
                g_bc[:],
                bass.AP(tensor=d_lng, offset=0, ap=[[0, 128], [1, H]]),
            )
            b_bc = consts.tile([128, H], F32)
            nc.gpsimd.dma_start(
                b_bc[:],
                bass.AP(tensor=d_lnb, offset=0, ap=[[0, 128], [1, H]]),
            )

        rep_ctx = tc.For_i(0, reps, 1) if reps > 1 else None
        if rep_ctx is not None:
            rep_ctx.__enter__()
        # ---- per-batch loads, issued for BOTH elements up front so stores
        # (later on the same queues) never delay the next element's loads.
        # SP queue: cT,cbf; ACT queue: qT,qn. First-needed tensors first.
        loads = {}
        for b in range(BPC):
            cT = p_inbf.tile([128, HC, CL], BF16, tag="cT")
            nc.sync.dma_start(cT[:], d_cT.ap()[b].rearrange("(o p) i -> p o i", p=128))
            cbf = p_inbf.tile([128, PC, H], BF16, tag="cbf")
            nc.sync.dma_start(cbf[:], d_cbf.ap()[b].rearrange("(o p) h -> p o h", p=128))
            qT = p_inbf.tile([128, HC, QL], BF16, tag="qT")
            nc.scalar.dma_start(qT[:], d_qT.ap()[b].rearrange("(o p) j -> p o j", p=128))
            qn = p_inbf.tile([128, QC, H], BF16, tag="qn")
            nc.scalar.dma_start(qn[:], d_qn.ap()[b].rearrange("(o p) h -> p o h", p=128))
            mk = None
            if use_mask:
                mk = p_inbf.tile([128, PC, QL], F32, tag="mask")
                nc.gpsimd.dma_start(
                    mk[:], d_mask.ap()[b].rearrange("(o p) j -> p o j", p=128)
                )
            loads[b] = (cT, cbf, qT, qn, mk)

        for b in range(BPC):
            cT, cbf, qT, qn, mk = loads[b]

            if "front" not in skip_stages:
                # ---- s_q, pre-broadcast: sq_bc[p, j] = sum_h wq[h] q[j, h] ----
                sq_ps = ps_aux.tile([128, QL], F32, tag="aux")
                for hc in range(HC):
                    nc.tensor.matmul(
                        sq_ps[:], lhsT=wq_rep[:, hc], rhs=qT[:, hc],
                        start=(hc == 0), stop=(hc == HC - 1),
                    )
                sq_bc = p_small.tile([128, QL], F32, tag="sq_bc")
                nc.scalar.copy(sq_bc[:], sq_ps[:])

                # ---- c_scaled^T = cT * wcq (per-partition scalar per h-chunk) ----
                csT = p_work.tile([128, HC, CL], BF16, tag="csT")
                for hc in range(HC):
                    nc.scalar.mul(csT[:, hc], cT[:, hc], wcq_c[:, hc : hc + 1])

                # ---- s0 = cq0 (+ s_q bcast add) (+mask); rowmax; E = exp; rowsum ----
                E = p_work.tile([128, PC, QL], BF16, tag="E")
                nrm = p_small.tile([128, PC], F32, tag="nrm")   # -rowmax
                rs = p_small.tile([128, PC], F32, tag="rs")     # rowsum of E
                for ic in range(PC):
                    s0 = ps_mm.tile([128, QL], F32, tag="mm")
                    for hc in range(HC):
                        nc.tensor.matmul(
                            s0[:], lhsT=csT[:, hc, ic * 128 : (ic + 1) * 128],
                            rhs=qT[:, hc], start=(hc == 0), stop=(hc == HC - 1),
                        )
                    nc.vector.scalar_tensor_tensor(
                        out=s0[:], in0=s0[:], scalar=0.0, in1=sq_bc[:],
                        op0=ADD, op1=ADD,
                    )
                    if use_mask:
                        nc.vector.tensor_add(s0[:], s0[:], mk[:, ic])
                    if "softmax" in skip_stages:
                        continue
                    nc.vector.tensor_reduce(
                        out=nrm[:, ic : ic + 1], in_=s0[:], axis=X, op=MAXOP,
                        negate=True,
                    )
                    nc.scalar.activation(
                        out=E[:, ic], in_=s0[:], func=AF.Exp,
                        bias=nrm[:, ic : ic + 1], scale=1.0,
                        accum_out=rs[:, ic : ic + 1],
                    )

                # ---- b path, fully in column form ----
                # s_c columns: sc[i] = sum_h c[i,h] wc[h] via accumulating STT
                sc_tmp = p_small.tile([128, H], BF16, tag="sc_tmp")
                sc_col = p_small.tile([128, PC], F32, tag="sc_col")
                for ic in range(PC):
                    nc.vector.scalar_tensor_tensor(
                        out=sc_tmp[:], in0=cbf[:, ic], scalar=0.0, in1=wc_bc[:],
                        op0=ADD, op1=MULT, accum_out=sc_col[:, ic : ic + 1],
                    )
                # m = s_c + rowmax = s_c - nrm; softmax over all 512 rows
                m_cols = p_small.tile([128, PC], F32, tag="m_cols")
                nc.vector.tensor_tensor(m_cols[:], sc_col[:], nrm[:], op=SUB)
                eb_cols = p_small.tile([128, PC], F32, tag="eb_cols")
                erow = p_small.tile([128, 1], F32, tag="erow")
                nc.scalar.activation(
                    out=eb_cols[:], in_=m_cols[:], func=AF.Exp, bias=nb3_t[:],
                    scale=1.0, accum_out=erow[:],
                )
                eS = p_small.tile([128, 1], F32, tag="eS")
                nc.gpsimd.partition_all_reduce(eS[:], erow[:], channels=128, reduce_op=RADD)
                rS = p_small.tile([128, 1], F32, tag="rS")
                nc.vector.reciprocal(rS[:], eS[:])
                b_cols = p_small.tile([128, PC], BF16, tag="b_cols")
                nc.vector.tensor_scalar_mul(b_cols[:], eb_cols[:], rS[:])

                # ---- 1/rowsum, diag blocks, A^T = E^T * diag (transpose+normalize) ----
                rr = p_small.tile([128, PC], F32, tag="rr")
                diag = p_work.tile([128, PC, 128], BF16, tag="diag")
                for ic in range(PC):
                    nc.vector.reciprocal(rr[:, ic : ic + 1], rs[:, ic : ic + 1])
                    nc.vector.tensor_scalar_mul(diag[:, ic], id_bf[:], rr[:, ic : ic + 1])
                AT = p_work.tile([128, QC, CL], BF16, tag="AT")
                for jc in range(QC):
                    at_ps = ps_mm.tile([128, CL], F32, tag="mm")
                    for ic in range(PC):
                        nc.tensor.matmul(
                            at_ps[:, ic * 128 : (ic + 1) * 128],
                            lhsT=E[:, ic, jc * 128 : (jc + 1) * 128],
                            rhs=diag[:, ic], start=True, stop=True,
                        )
                    nc.scalar.copy(AT[:, jc], at_ps[:])

                # ---- c2q^T (h-part) + xc = (c*c2q)^T ----
                c2qT = p_xmat.tile([128, HC, CL], BF16, tag="c2qT")
                xc = p_xmat.tile([128, HC, CL], BF16, tag="xc")
                for hc in range(HC):
                    cq_ps = ps_mm.tile([128, CL], F32, tag="mm")
                    for jc in range(QC):
                        nc.tensor.matmul(
                            cq_ps[:], lhsT=qn[:, jc, hc * 128 : (hc + 1) * 128],
                            rhs=AT[:, jc], start=(jc == 0), stop=(jc == QC - 1),
                        )
                    nc.scalar.copy(c2qT[:, hc], cq_ps[:])
                    nc.vector.tensor_tensor(
                        xc[:, hc], cT[:, hc], c2qT[:, hc], op=MULT
                    )

                # ---- q2c row = b @ c  -> columns (h-part) ----
                q2c_sb = p_small.tile([1, H], F32, tag="q2c_sb")
                for n0, nw in ((0, 512), (512, 256)):
                    qp = ps_aux.tile([1, nw], F32, tag="aux")
                    for ic in range(PC):
                        nc.tensor.matmul(
                            qp[:],
                            lhsT=b_cols[:, ic : ic + 1],
                            rhs=cbf[:, ic, n0 : n0 + nw],
                            start=(ic == 0), stop=(ic == PC - 1),
                        )
                    nc.scalar.copy(q2c_sb[0:1, n0 : n0 + nw], qp[:])
                qcc_ps = ps_aux.tile([128, HC], F32, tag="aux")
                for hc in range(HC):
                    nc.tensor.matmul(
                        qcc_ps[:, hc : hc + 1],
                        lhsT=q2c_sb[0:1, hc * 128 : (hc + 1) * 128],
                        rhs=id_f32[0:1, 0:1], start=True, stop=True,
                    )
                q2c_c = p_small.tile([128, HC], F32, tag="q2c_c")
                nc.scalar.copy(q2c_c[:], qcc_ps[:])
                # Fold the (c*q2c) concat component into the c-component weights:
                #   sum_f cT[f,i]*q2c[f]*wa4T[f,ho] == c @ (diag(q2c) wa4T)
                # so big-matmul uses merged = wa1T + q2c (.) wa4T for comp 0.
                merged = p_work.tile([128, HC, H], BF16, tag="merged")
                for hc in range(HC):
                    nc.vector.scalar_tensor_tensor(
                        out=merged[:, hc], in0=waT[:, 3 * HC + hc],
                        scalar=q2c_c[:, hc : hc + 1], in1=waT[:, hc],
                        op0=MULT, op1=ADD,
                    )

            if "big" not in skip_stages:
                # ---- big matmul: y0 = x @ wa^T; +bias; relu; +c; layernorm ----
                if "front" in skip_stages:
                    comps = ((cT, 0, None), (cT, 1, None), (cT, 2, None))
                else:
                    # (tile, waT comp index, rhs override); merged last — its
                    # weights depend on the b-path and arrive latest.
                    comps = ((c2qT, 1, None), (xc, 2, None), (cT, 0, merged))
                yt = p_y.tile([128, PC, H], F32, tag="y")
                for ic in range(PC):
                    big_ps = ps_big.tile([128, H], F32, tag="big")
                    k = 0
                    for comp, wci, rhs_src in comps:
                        for hc in range(HC):
                            rhs3 = (
                                rhs_src[:, hc] if rhs_src is not None
                                else waT[:, wci * HC + hc]
                            )
                            for n0, nw in ((0, 512), (512, 256)):
                                nc.tensor.matmul(
                                    big_ps[:, n0 : n0 + nw],
                                    lhsT=comp[:, hc, ic * 128 : (ic + 1) * 128],
                                    rhs=rhs3[:, n0 : n0 + nw],
                                    start=(k == 0), stop=(k == 3 * HC - 1),
                                    skip_group_check=True,
                                )
                            k += 1
                    if "epilogue" in skip_stages:
                        continue
                    nc.vector.scalar_tensor_tensor(
                        out=big_ps[:], in0=big_ps[:], scalar=0.0, in1=wab_bc[:],
                        op0=ADD, op1=ADD,
                    )
                    nc.vector.scalar_tensor_tensor(
                        out=yt[:, ic], in0=big_ps[:], scalar=0.0,
                        in1=cbf[:, ic], op0=MAXOP, op1=ADD,
                    )
                    stats = p_small.tile([128, 3, 6], F32, tag="stats")
                    for sg in range(3):
                        nc.vector.bn_stats(
                            out=stats[:, sg], in_=yt[:, ic, sg * 256 : (sg + 1) * 256]
                        )
                    mv = p_small.tile([128, 2], F32, tag="mv")
                    nc.vector.bn_aggr(out=mv[:], in_=stats[:])
                    lnv = p_small.tile([128, 1], F32, tag="lnv")
                    nc.scalar.activation(
                        out=lnv[:], in_=mv[:, 1:2], func=AF.Ln, bias=eps_t[:], scale=1.0
                    )
                    rstd = p_small.tile([128, 1], F32, tag="rstd")
                    nc.scalar.activation(out=rstd[:], in_=lnv[:], func=AF.Exp, scale=-0.5)
                    nc.vector.tensor_scalar(
                        out=yt[:, ic], in0=yt[:, ic],
                        scalar1=mv[:, 0:1], scalar2=rstd[:],
                        op0=SUB, op1=MULT,
                    )
                    if not trivial_ln:
                        nc.vector.tensor_tensor(yt[:, ic], yt[:, ic], g_bc[:], op=MULT)
                        nc.vector.tensor_add(yt[:, ic], yt[:, ic], b_bc[:])
                    nc.sync.dma_start(
                        d_out.ap()[b].rearrange("(o p) h -> p o h", p=128)[:, ic], yt[:, ic]
                    )

        if rep_ctx is not None:
            rep_ctx.__exit__(None, None, None)

    nc.compile()
    return nc


_KERNEL_CACHE = {}


def get_kernel(use_mask: bool, trivial_ln: bool):
    key = (use_mask, trivial_ln)
    if key not in _KERNEL_CACHE:
        _KERNEL_CACHE[key] = build_kernel(use_mask, trivial_ln)
    return _KERNEL_CACHE[key]


def prep_inputs(inputs):
    """Host-side layout prep: shard over batch, transpose/cast, weight reshape."""
    c = np.ascontiguousarray(np.asarray(inputs["inputs"], dtype=np.float32))
    q = np.ascontiguousarray(np.asarray(inputs["states"], dtype=np.float32))
    mask = np.asarray(inputs["attention_mask"], dtype=np.float32)[:, 0]
    use_mask = bool(np.any(mask))
    ln_g = np.asarray(inputs["ln_g"], dtype=np.float32)
    ln_b = np.asarray(inputs["ln_b"], dtype=np.float32)
    trivial_ln = bool(np.all(ln_g == 1.0) and np.all(ln_b == 0.0))

    cbf = c.astype(BF)
    qn = q.astype(BF)
    cT = np.ascontiguousarray(c.transpose(0, 2, 1)).astype(BF)
    qT = np.ascontiguousarray(q.transpose(0, 2, 1)).astype(BF)

    wcq_cols = np.ascontiguousarray(
        np.asarray(inputs["wcq_w"], np.float32)[0].reshape(HC, 128).T
    )
    wq_cols = np.ascontiguousarray(
        np.asarray(inputs["wq_w"], np.float32)[0].reshape(HC, 128).T
    )
    wc_row = np.asarray(inputs["wc_w"], np.float32).reshape(1, H).astype(BF)
    waT = np.ascontiguousarray(
        np.asarray(inputs["wa_w"], np.float32).T.reshape(FC, 128, H).transpose(1, 0, 2)
    ).astype(BF)
    wab = np.asarray(inputs["wa_b"], np.float32).reshape(1, H)

    in_maps = []
    for k in range(N_CORES):
        sl = slice(k * BPC, (k + 1) * BPC)
        m = {
            "cbf": cbf[sl],
            "cT": cT[sl],
            "qT": qT[sl],
            "qn": qn[sl],
            "wcq": wcq_cols,
            "wc": wc_row,
            "wq": wq_cols,
            "waT": waT,
            "wab": wab,
        }
        if use_mask:
            m["mask"] = np.ascontiguousarray(mask[sl])
        if not trivial_ln:
            m["lng"] = ln_g
            m["lnb"] = ln_b
        in_maps.append(m)
    return in_maps, use_mask, trivial_ln


def kernel(**inputs) -> np.ndarray:
    in_maps, use_mask, trivial_ln = prep_inputs(inputs)
    nc = get_kernel(use_mask, trivial_ln)
    res = run_bass_kernel_spmd(nc, in_maps, core_ids=list(range(N_CORES)))
    return np.concatenate([res.results[k]["out"] for k in range(N_CORES)], axis=0)
